# revision 8
# baseline (speedup 1.0000x reference)
"""DLRM DotInteractionArch kernel for 8x Trainium2 NeuronCores — v6.

Problem: B=16384, 26 sparse embeddings + 1 dense feature, D=128.
  combined[b] = concat(dense[b], emb[b])           # [27, 128]
  G[b] = combined[b] @ combined[b].T               # [27, 27]
  out[b] = concat(dense[b], triu(G[b], k=1).flat)  # [479]

v4: v3's wall was PE instruction count (fixed ~50ns/matmul + ~15ns+P/f
per LDWEIGHTS, 181 matmul pairs/round). v4 cuts to 85 pairs/round:
  - transposes: 27/round (unchanged) -> CTS2[128 d, 128 b, 32 f-pad] bf16
    (b-major so the slab stationary is contiguous for fast weight load)
  - slab-G: ONE matmul per 4 samples: lhsT = rhs = CTS2[:, 4k:4k+4, :]
    ([128 d, 128 = 4 samples x 32 f-slots]); out[32si+f, 32sj+g] in PSUM
    holds the four G blocks on the diagonal (si==sj). 32 matmuls/round.
  - evac slab-G -> GP_sb[128 p, 128 (sj,f'), 4 r4, 32 k] bf16 with the
    (r4, k) sample index innermost-contiguous, batching 4 rounds.
  - pack (every 4 rounds): for (f, si): lhsT = GP_sb[32si:32si+27,
    32si+f, :, :] ([27 g, 128 samples] CONTIGUOUS -> FWL), rhs =
    IDENT4[32si:32si+27, f+1:27], accumulated into PK_si[128, 351]
    (tile_position=(32si, 0)). By G's symmetry partition 32si+g / column
    32si+f holds G[f, g]. 104 matmuls per 4 rounds = 26/round.
  - output rows for strip si of a 4-round block are samples
    b = rb*512 + 4*i + si (i = PSUM partition) — a plain stride-4 row
    interleave absorbed by the store/dense-load APs. One store DMA per
    (block, si): [128, 479] fp32, 1916B descriptors.

Sample mapping per core: b = rb*512 + i*4 + si.
"""

import numpy as np

B_FULL = 16384
N_CORES = 8
BC = B_FULL // N_CORES  # 2048 samples per core
F = 27                  # 1 dense + 26 sparse features
D = 128
NSPARSE = 26
PAIRS = F * (F - 1) // 2  # 351
OUTC = D + PAIRS          # 479
ROUNDS = BC // 128        # 16

_CACHE = {}


def _triu_offsets():
    off = [D]
    for f in range(F - 1):
        off.append(off[-1] + (NSPARSE - f))
    return off


def _build_nc(bc: int = BC, debug_init: bool = False):
    from contextlib import ExitStack

    import concourse.bacc as bacc
    import concourse.tile as tile
    from concourse import mybir

    BF = mybir.dt.bfloat16
    F32 = mybir.dt.float32
    rounds = bc // 128
    blocks = rounds // 4

    nc = bacc.Bacc("TRN2", target_bir_lowering=False, debug=False)
    den = nc.dram_tensor("dense_output", [bc, D], F32, kind="ExternalInput")
    emb = nc.dram_tensor("embeddings", [bc, NSPARSE, D], F32, kind="ExternalInput")
    out = nc.dram_tensor("out", [bc, OUTC], F32, kind="ExternalOutput")

    off = _triu_offsets()

    emb_v = emb.ap().rearrange("(r p) j d -> r p j d", p=128)
    den_v = den.ap().rearrange("(r p) d -> p r d", p=128)       # bf16 preload
    den_v3 = den.ap().rearrange("(rb i s) d -> rb s i d", i=128, s=4)
    out_v3 = out.ap().rearrange("(rb i s) c -> rb s i c", i=128, s=4)

    chunks = [list(range(8 * c, min(8 * c + 8, F))) for c in range(4)]

    with tile.TileContext(nc) as tc, ExitStack() as ctx:
        const = ctx.enter_context(tc.tile_pool(name="const", bufs=1))
        lp = ctx.enter_context(tc.tile_pool(name="l", bufs=4))
        ctsp = ctx.enter_context(tc.tile_pool(name="cts", bufs=3))
        gsp = ctx.enter_context(tc.tile_pool(name="gs", bufs=2))
        outp = ctx.enter_context(tc.tile_pool(name="o", bufs=8))
        ptp = ctx.enter_context(tc.tile_pool(name="pt", bufs=3, space="PSUM"))
        gpp = ctx.enter_context(tc.tile_pool(name="gp", bufs=3, space="PSUM"))
        pkp = ctx.enter_context(tc.tile_pool(name="pk", bufs=2, space="PSUM"))

        # 4-block identity: IDENT4[32*si + g, c] = 1 iff g == c (g < 32)
        ident4 = const.tile([128, 32], BF)
        nc.gpsimd.memset(ident4[:], 0.0)
        for si in range(4):
            nc.gpsimd.affine_select(
                out=ident4[32 * si:32 * si + 32, :],
                in_=ident4[32 * si:32 * si + 32, :],
                compare_op=mybir.AluOpType.not_equal,
                fill=1.0,
                base=0,
                pattern=[[-1, 32]],
                channel_multiplier=1,
            )
        # full 128 identity for the PE transposes
        ident = const.tile([128, 128], BF)
        nc.gpsimd.memset(ident[:], 0.0)
        nc.gpsimd.affine_select(
            out=ident[:],
            in_=ident[:],
            compare_op=mybir.AluOpType.not_equal,
            fill=1.0,
            base=0,
            pattern=[[-1, 128]],
            channel_multiplier=1,
        )

        # dense rows for all rounds (bf16, for the G stage); loaded per-round
        # inside the loop so round 0's transposes wait only on two small DMAs
        D_all = const.tile([128, rounds, D], BF)

        for rb in range(blocks):
            GP_sb = gsp.tile([128, 4, 32, 128], BF)  # [p, r4, k, (sj,f')]
            for r4 in range(4):
                r = rb * 4 + r4
                # ---- load ----
                L = lp.tile([128, NSPARSE, D], BF)
                # split so transpose chunk 0 (features 0..7 -> L[0:7]) can
                # start before the rest of the round's load lands
                nc.gpsimd.dma_start(out=L[:, 0:7], in_=emb_v[r][:, 0:7])
                nc.gpsimd.dma_start(
                    out=D_all[:, r], in_=den.ap()[r * 128:(r + 1) * 128, :]
                )
                for lo, hi in ((7, 15), (15, 23), (23, NSPARSE)):
                    nc.gpsimd.dma_start(
                        out=L[:, lo:hi], in_=emb_v[r][:, lo:hi]
                    )

                # ---- transposes -> CTS2[128 d, 128 b, 32 f] ----
                CTS2 = ctsp.tile([128, 128, 32], BF)
                if debug_init:
                    nc.vector.memset(CTS2[:], 0.0)
                for ci, chunk in enumerate(chunks):
                    PT = ptp.tile([128, len(chunk), 128], BF)
                    for kk, f in enumerate(chunk):
                        src = D_all[:, r, :] if f == 0 else L[:, f - 1, :]
                        nc.tensor.transpose(PT[:, kk], src, ident[:])
                    # split each evac across DVE+ACT (b halves) to halve latency
                    for bh, eng_copy in (
                        (0, nc.vector.tensor_copy),
                        (1, nc.scalar.copy),
                    ):
                        bs = slice(64 * bh, 64 * bh + 64)
                        if len(chunk) % 2 == 0:
                            dst = CTS2[:, bs, chunk[0]:chunk[-1] + 1].rearrange(
                                "d b (fp f2) -> d fp b f2", f2=2
                            )
                            srcp = PT[:, :, bs].rearrange(
                                "d (fp f2) b -> d fp b f2", f2=2
                            )
                        else:
                            dst = CTS2[:, bs, chunk[0]:chunk[-1] + 1].rearrange(
                                "d b f -> d f b"
                            )
                            srcp = PT[:, :, bs]
                        eng_copy(dst, srcp)

                # ---- slab-G: 32 matmuls, 4 samples each ----
                for kq in range(8):  # 4 slabs per PSUM bank-tile
                    GPp = gpp.tile([128, 4, 128], F32)
                    for j in range(4):
                        k = kq * 4 + j
                        slab = CTS2[:, 4 * k:4 * k + 4, :].rearrange(
                            "d s f -> d (s f)"
                        )
                        nc.tensor.matmul(
                            GPp[:, j, :], slab, slab, start=True, stop=True
                        )
                    # evac: dst[p, (sj,f'), r4, k] <- src[p, k-slab, (sj,f')]
                    # split across DVE+ACT to halve latency
                    nc.vector.tensor_copy(
                        GP_sb[:, r4, 4 * kq:4 * kq + 2, :], GPp[:, 0:2]
                    )
                    nc.scalar.copy(
                        GP_sb[:, r4, 4 * kq + 2:4 * kq + 4, :], GPp[:, 2:4]
                    )

            # ---- pack: 104 matmuls per block -> 4x PK[128, 351] ----
            for si in range(4):
                PK = pkp.tile([128, 512], F32)
                for f in range(F - 1):
                    n = NSPARSE - f
                    lhsT = GP_sb[32 * si:32 * si + 27, :, :, 32 * si + f].rearrange(
                        "g r k -> g (r k)"
                    )
                    nc.tensor.matmul(
                        PK[:, off[f] - D:off[f] - D + n],
                        lhsT,
                        ident4[32 * si:32 * si + 27, f + 1:F],
                        start=(f == 0),
                        stop=(f == F - 2),
                        tile_position=(32 * si, 0),
                    )
                OUT = outp.tile([128, OUTC], F32)
                nc.scalar.dma_start(out=OUT[:, 0:D], in_=den_v3[rb][si])
                nc.vector.tensor_copy(
                    OUT[:, D:D + 176], PK[:, 0:176]
                )
                nc.scalar.copy(
                    OUT[:, D + 176:OUTC], PK[:, 176:PAIRS]
                )
                st_eng = nc.sync if si % 2 == 0 else nc.scalar
                st_eng.dma_start(out=out_v3[rb][si], in_=OUT[:])

    nc.finalize()
    return nc


def kernel(dense_output: np.ndarray, embeddings: np.ndarray) -> np.ndarray:
    from concourse.bass_utils import run_bass_kernel_spmd

    if "nc" not in _CACHE:
        _CACHE["nc"] = _build_nc()
    nc = _CACHE["nc"]

    dense_output = np.ascontiguousarray(np.asarray(dense_output, dtype=np.float32))
    embeddings = np.ascontiguousarray(np.asarray(embeddings, dtype=np.float32))
    in_maps = []
    for i in range(N_CORES):
        sl = slice(i * BC, (i + 1) * BC)
        in_maps.append(
            {
                "dense_output": np.ascontiguousarray(dense_output[sl]),
                "embeddings": np.ascontiguousarray(embeddings[sl]),
            }
        )
    res = run_bass_kernel_spmd(nc, in_maps, list(range(N_CORES)))
    return np.concatenate([res.results[i]["out"] for i in range(N_CORES)], axis=0)
